# revision 1
# baseline (speedup 1.0000x reference)
"""CosineAttention on 8 TRN2 NeuronCores.

Sharding (head + tensor parallel, per the hint):
  core c owns head h=c for both batches:
    - computes qT,kT = [Wq_h|Wk_h]^T-stationary matmuls over full xT
    - RMS-normalizes q,k in the [d, i] layout via a PE ones-matmul
      partition-reduction + K=2 outer-product broadcast
    - simT[j,i] = kn^T qn (K=64, float32r), exp folded scale=1/8 on ACT
    - attn@v with a ones column appended to v so the softmax denominator
      falls out as matmul row 64; normalize by its reciprocal
    - per-batch AllGather of o_cT [64, n] (bf16) -> [512, n] feature-major
    - column-parallel out-proj: outT_c [64, n] = W2_c^T-stationary matmul
  host concatenates the 8 feature slices.

Matmul dtype: float32r (TF32-class single-pass PE mode) for the fp32 path;
bf16 for the post-softmax path (attn weights / v / out-proj operands).
"""

import numpy as np
import ml_dtypes

import concourse.bass as bass
import concourse.tile as tile
from concourse import bacc
import concourse.mybir as mybir
from concourse import bass_utils

f32 = mybir.dt.float32
f32r = mybir.dt.float32r
bf16 = mybir.dt.bfloat16
AF = mybir.ActivationFunctionType

N_CORES = 8
HEADS = 8
D = 64            # head dim
B = 2             # batch
SEQ = 2048        # tokens per batch
DIM = 512         # model dim = HEADS * D
NTOK = B * SEQ    # 4096
EPS = 1e-4
SCALE = D ** -0.5  # 0.125

FT = DIM // 128   # 4 f-tiles of 128
CH1 = 512         # stage-1 token chunk
NCH1 = NTOK // CH1            # 8
ICH = 1024        # phase-2 i-chunk (exp batching)
NICH = SEQ // ICH             # 2 per batch
JT = SEQ // 128   # 16 j-tiles per batch
PCH = 512         # phase-3 chunk
NPCH = SEQ // PCH             # 4 per batch

_BUILD_CACHE = {}


def build(collective=True, num_devices=N_CORES, reps=1):
    key = (collective, num_devices, reps)
    if key in _BUILD_CACHE:
        return _BUILD_CACHE[key]
    nc = bacc.Bacc("TRN2", target_bir_lowering=False, debug=False,
                   num_devices=num_devices)
    xT = nc.dram_tensor("xT", [DIM, NTOK], f32, kind="ExternalInput").ap()
    wqk = nc.dram_tensor("wqk", [DIM, 128], f32, kind="ExternalInput").ap()
    wv = nc.dram_tensor("wv", [DIM, D], f32, kind="ExternalInput").ap()
    w2 = nc.dram_tensor("w2", [DIM, D], bf16, kind="ExternalInput").ap()
    ones2 = nc.dram_tensor("ones2", [128, 2], f32, kind="ExternalInput").ap()
    e2 = nc.dram_tensor("e2", [2, 128], f32, kind="ExternalInput").ap()
    ones1 = nc.dram_tensor("ones1", [1, D], f32, kind="ExternalInput").ap()
    outT = nc.dram_tensor("outT", [D, NTOK], f32, kind="ExternalOutput").ap()

    with tile.TileContext(nc) as tc:
        with (
            tc.tile_pool(name="persist", bufs=1) as pp,
            tc.tile_pool(name="sb", bufs=2) as sb,
            tc.tile_pool(name="ps", bufs=1, space="PSUM") as ps,
            tc.tile_pool(name="dram", bufs=1, space="DRAM") as dram,
            nc.allow_low_precision(reason="f32r matmuls; bf16 attn/out path"),
        ):
            # ---- persistent weights / constants ----
            wqk_sb = pp.tile([128, FT, 128], f32r)
            wv_sb = pp.tile([128, FT, D], f32r)
            w2_sb = pp.tile([128, FT, D], bf16)
            for t in range(FT):
                nc.sync.dma_start(wqk_sb[:, t, :],
                                  wqk[t * 128:(t + 1) * 128, :].bitcast(f32r))
                nc.sync.dma_start(wv_sb[:, t, :],
                                  wv[t * 128:(t + 1) * 128, :].bitcast(f32r))
                nc.sync.dma_start(w2_sb[:, t, :], w2[t * 128:(t + 1) * 128, :])
            o2_sb = pp.tile([128, 2], f32r)
            nc.sync.dma_start(o2_sb[:], ones2[:].bitcast(f32r))
            e2_sb = pp.tile([2, 128], f32r)
            nc.sync.dma_start(e2_sb[:], e2[:].bitcast(f32r))
            o1_sb = pp.tile([1, D], f32r)
            nc.sync.dma_start(o1_sb[:], ones1[:].bitcast(f32r))

            # ---- persistent activations ----
            qn_sb = pp.tile([D, NTOK], f32r)     # normalized qT
            kn_sb = pp.tile([D, NTOK], f32r)     # normalized kT
            qk_all = pp.tile([128, NTOK], f32r)  # raw [q;k]T
            vo_sb = pp.tile([128, NTOK // 128, D + 1], bf16)  # v | ones

            # ---- stage 1: qkv projections + rms normalization ----
            for rep in range(reps):
              for ci in range(NCH1):
                  cols = slice(ci * CH1, (ci + 1) * CH1)
                  xt_sb = sb.tile([128, FT, CH1], f32r, tag="xt")
                  for t in range(FT):
                      nc.sync.dma_start(
                          xt_sb[:, t, :],
                          xT[t * 128:(t + 1) * 128, cols].bitcast(f32r))

                  # qkT chunk: [q;k] x-stream, W stationary
                  qk_ps = ps.tile([128, CH1], f32, tag="small", bufs=4)
                  for t in range(FT):
                      nc.tensor.matmul(qk_ps[:], wqk_sb[:, t, :], xt_sb[:, t, :],
                                       start=(t == 0), stop=(t == FT - 1))
                  # v chunk: [j, d], xT stationary
                  for js in range(CH1 // 128):
                      v_ps = ps.tile([128, D], f32, tag="small", bufs=4)
                      for t in range(FT):
                          nc.tensor.matmul(
                              v_ps[:],
                              xt_sb[:, t, js * 128:(js + 1) * 128],
                              wv_sb[:, t, :],
                              start=(t == 0), stop=(t == FT - 1))
                      jt = ci * (CH1 // 128) + js
                      nc.vector.tensor_copy(vo_sb[:, jt, 0:D], v_ps[:])
                      nc.gpsimd.memset(vo_sb[:, jt, D:D + 1], 1.0)

                  # raw qk to sbuf (ACT; DVE is busier)
                  nc.scalar.activation(qk_all[:, cols], qk_ps[:], AF.Copy)
                  # sq = qk^2 (DVE, from the sbuf copy)
                  sq_sb = sb.tile([128, CH1], f32r, tag="sq")
                  nc.vector.tensor_mul(sq_sb[:], qk_all[:, cols],
                                       qk_all[:, cols])
                  # st[2, CH1] = column sums of q-sq and k-sq
                  st_ps = ps.tile([2, CH1], f32, tag="small", bufs=4)
                  nc.tensor.matmul(st_ps[:], o2_sb[:], sq_sb[:],
                                   start=True, stop=True)
                  # r = 1/(sqrt(st/64) + eps)
                  rt_sb = sb.tile([2, CH1], f32, tag="rt")
                  nc.scalar.activation(rt_sb[:], st_ps[:], AF.Sqrt,
                                       scale=1.0 / D)
                  re_sb = sb.tile([2, CH1], f32, tag="re")
                  nc.vector.tensor_scalar_add(re_sb[:], rt_sb[:], EPS)
                  rc_sb = sb.tile([2, CH1], f32r, tag="rc")
                  nc.vector.reciprocal(rc_sb[:], re_sb[:])
                  # R[128, CH1] = outer(e2, r): row broadcast of scales
                  r_ps = ps.tile([128, CH1], f32, tag="small", bufs=4)
                  nc.tensor.matmul(r_ps[:], e2_sb[:], rc_sb[:],
                                   start=True, stop=True)
                  rb_sb = sb.tile([128, CH1], f32r, tag="rb")
                  nc.scalar.activation(rb_sb[:], r_ps[:], AF.Copy)
                  # apply
                  nc.vector.tensor_mul(qn_sb[:, cols], qk_all[0:D, cols],
                                       rb_sb[0:D, :])
                  nc.vector.tensor_mul(kn_sb[:, cols], qk_all[D:128, cols],
                                       rb_sb[D:128, :])

              # ---- per-batch: attention + allgather + out-proj ----
              cc_outs = []
              for b in range(B):
                  cc_in = dram.tile([D, SEQ], bf16, name=f"cc_in{b}")
                  cc_out = dram.tile([DIM, SEQ], bf16, addr_space="Shared",
                                     name=f"cc_out{b}")
                  cc_outs.append(cc_out)
                  for ic in range(NICH):
                      i0 = b * SEQ + ic * ICH
                      expT = sb.tile([128, JT, ICH], bf16, tag="expT")
                      for jt in range(JT):
                          j0 = b * SEQ + jt * 128
                          sim_ps = ps.tile([128, ICH], f32, tag="big", bufs=2)
                          for h in range(ICH // 512):
                              nc.tensor.matmul(
                                  sim_ps[:, h * 512:(h + 1) * 512],
                                  kn_sb[:, j0:j0 + 128],
                                  qn_sb[:, i0 + h * 512:i0 + (h + 1) * 512],
                                  start=True, stop=True)
                          nc.scalar.activation(expT[:, jt, :], sim_ps[:],
                                               AF.Exp, scale=SCALE)
                      for h in range(ICH // 512):
                          av_ps = ps.tile([D + 1, 512], f32, tag="small", bufs=4)
                          for jt in range(JT):
                              nc.tensor.matmul(
                                  av_ps[:],
                                  vo_sb[:, b * JT + jt, :],
                                  expT[:, jt, h * 512:(h + 1) * 512],
                                  start=(jt == 0), stop=(jt == JT - 1))
                          # normalize by sumexp (row D) and emit bf16
                          rse_sb = sb.tile([1, 512], f32r, tag="rse")
                          nc.vector.reciprocal(rse_sb[:],
                                               av_ps[D:D + 1, :].bitcast(f32r))
                          r2_ps = ps.tile([D, 512], f32, tag="small", bufs=4)
                          nc.tensor.matmul(r2_ps[:], o1_sb[:], rse_sb[:],
                                           start=True, stop=True)
                          r2_sb = sb.tile([D, 512], f32, tag="r2")
                          nc.scalar.activation(r2_sb[:], r2_ps[:], AF.Copy)
                          oc_sb = sb.tile([D, 512], bf16, tag="oc")
                          nc.vector.tensor_mul(oc_sb[:], av_ps[0:D, :], r2_sb[:])
                          nc.sync.dma_start(
                              cc_in[:, ic * ICH + h * 512:
                                    ic * ICH + (h + 1) * 512], oc_sb[:])
                  if collective:
                      nc.gpsimd.collective_compute(
                          "AllGather", mybir.AluOpType.bypass,
                          replica_groups=[list(range(num_devices))],
                          ins=[cc_in[:]], outs=[cc_out[:]])
                  else:
                      # timing-only stand-in: keep the DRAM write traffic
                      nc.sync.dma_start(cc_out[0:D, :], cc_in[:])

              for b in range(B):
                  cc_out = cc_outs[b]
                  for pc in range(NPCH):
                      cols = slice(pc * PCH, (pc + 1) * PCH)
                      ag_sb = sb.tile([128, FT, PCH], bf16, tag="ag")
                      for t in range(FT):
                          nc.sync.dma_start(ag_sb[:, t, :],
                                            cc_out[t * 128:(t + 1) * 128, cols])
                      fp_ps = ps.tile([D, PCH], f32, tag="small", bufs=4)
                      for t in range(FT):
                          nc.tensor.matmul(fp_ps[:], w2_sb[:, t, :],
                                           ag_sb[:, t, :],
                                           start=(t == 0), stop=(t == FT - 1))
                      fo_sb = sb.tile([D, PCH], f32, tag="fo")
                      nc.vector.tensor_copy(fo_sb[:], fp_ps[:])
                      nc.sync.dma_start(
                          outT[:, b * SEQ + pc * PCH:b * SEQ + (pc + 1) * PCH],
                          fo_sb[:])
    nc.compile()
    _BUILD_CACHE[key] = nc
    return nc


def make_in_maps(x, Wq, Wkv, Wout):
    xT = np.ascontiguousarray(x.reshape(NTOK, DIM).T).astype(np.float32)
    ones2 = np.zeros((128, 2), np.float32)
    ones2[0:D, 0] = 1.0
    ones2[D:128, 1] = 1.0
    e2 = np.ascontiguousarray(ones2.T)
    ones1 = np.ones((1, D), np.float32)
    in_maps = []
    for c in range(N_CORES):
        rows = slice(c * D, (c + 1) * D)
        wqk = np.ascontiguousarray(
            np.concatenate([Wq[rows, :].T, Wkv[rows, :].T], axis=1))
        wv = np.ascontiguousarray(Wkv[DIM + c * D:DIM + (c + 1) * D, :].T)
        w2 = np.ascontiguousarray(Wout[rows, :].T).astype(ml_dtypes.bfloat16)
        in_maps.append({
            "xT": xT, "wqk": wqk.astype(np.float32),
            "wv": wv.astype(np.float32), "w2": w2,
            "ones2": ones2, "e2": e2, "ones1": ones1,
        })
    return in_maps


def kernel(x, Wq, Wkv, Wout, _trace=False):
    nc = build()
    in_maps = make_in_maps(np.asarray(x), np.asarray(Wq), np.asarray(Wkv),
                           np.asarray(Wout))
    res = bass_utils.run_bass_kernel_spmd(
        nc, in_maps, core_ids=list(range(N_CORES)), trace=_trace)
    out = np.empty((NTOK, DIM), np.float32)
    for c in range(N_CORES):
        out[:, c * D:(c + 1) * D] = res.results[c]["outT"].T
    full = out.reshape(B, SEQ, DIM)
    if _trace:
        return full, res
    return full



# revision 10
# speedup vs baseline: 1.0183x; 1.0183x over previous
"""CosineAttention on 8 TRN2 NeuronCores.

Sharding: head-parallel attention + AllToAll shard-transpose +
token-parallel out-projection.

  core c owns head h=c for both batches:
    - stage 1 (per 512-token chunk): [q;k]T and vT via weight-stationary
      bf16 matmuls over xT; vT DMA-XBAR-transposed into packed [j, 64]
      tiles then packed into [j, 65] (ones col 64); k remapped to
      partitions 0-63; per-j k sum-of-squares via ones-matmul;
      per-token q sum-of-squares via ones-matmul.
    - stage 1.5: batched reciprocals: rq = 1/(sqrt(mean q^2)+eps) for all
      4096 tokens in one instruction; rk = 1/(sqrt(sum k^2)+8eps) per j
      (folds SCALE=1/8); qn = q * broadcast(rq).
    - phase 2 (per 512-token i-chunk): simT[j,i] = kraw^T qn; exp on ACT
      with per-partition scale AP rk[j]; attn@[v|1] accumulates so row 64
      is the softmax denominator Z; oc = av[0:64] / Z in bf16.
    - ONE AllToAll [512, 512] bf16: shard g = [64, 512] feature tile for
      token block g -> core receives all 512 features for its own 512
      tokens.
    - token-parallel out-proj with the full Wout (16 matmuls) -> outT
      [512 features, 512 tokens] f32; host concatenates token blocks.
"""

import numpy as np
import ml_dtypes

import concourse.bass as bass
import concourse.tile as tile
from concourse import bacc
import concourse.mybir as mybir
from concourse import bass_utils

f32 = mybir.dt.float32
f32r = mybir.dt.float32r
bf16 = mybir.dt.bfloat16
AF = mybir.ActivationFunctionType
ALU = mybir.AluOpType

N_CORES = 8
HEADS = 8
D = 64            # head dim
B = 2             # batch
SEQ = 2048        # tokens per batch
DIM = 512         # model dim
NTOK = B * SEQ    # 4096
EPS = 1e-4
SCALE = D ** -0.5  # 0.125

CH = 512          # token chunk = A2A shard = out-proj block
NCH = NTOK // CH  # 8
JPC = CH // 128   # 4 j-tiles per chunk
JPB = SEQ // 128  # 16 j-tiles per batch

_BUILD_CACHE = {}


def build(num_devices=N_CORES, collective=True):
    key = (num_devices, collective)
    if key in _BUILD_CACHE:
        return _BUILD_CACHE[key]
    nc = bacc.Bacc("TRN2", target_bir_lowering=False, debug=False,
                   num_devices=num_devices)
    xT = nc.dram_tensor("xT", [DIM, NTOK], bf16, kind="ExternalInput").ap()
    wqk = nc.dram_tensor("wqk", [DIM, 128], bf16, kind="ExternalInput").ap()
    wv = nc.dram_tensor("wv", [DIM, D], bf16, kind="ExternalInput").ap()
    w2 = nc.dram_tensor("w2", [DIM, DIM], bf16, kind="ExternalInput").ap()
    o64 = nc.dram_tensor("o64", [D, 1], bf16, kind="ExternalInput").ap()
    onr = nc.dram_tensor("onr", [1, 128], bf16, kind="ExternalInput").ap()
    outT = nc.dram_tensor("outT", [DIM, CH], f32, kind="ExternalOutput").ap()

    with tile.TileContext(nc) as tc:
        with (
            tc.tile_pool(name="persist", bufs=1) as pp,
            tc.tile_pool(name="sb", bufs=2) as sb,
            tc.tile_pool(name="ps", bufs=1, space="PSUM") as ps,
            tc.tile_pool(name="dram", bufs=1, space="DRAM") as dram,
            nc.allow_low_precision(reason="bf16 matmul path"),
        ):
            # ---- persistent weights / constants ----
            wqk_sb = pp.tile([128, 4, 128], bf16)
            wv_sb = pp.tile([128, 4, D], bf16)
            w2_sb = pp.tile([128, 4, DIM], bf16)
            for t in range(4):
                nc.sync.dma_start(wqk_sb[:, t, :], wqk[t * 128:(t + 1) * 128, :])
                nc.sync.dma_start(wv_sb[:, t, :], wv[t * 128:(t + 1) * 128, :])
                nc.sync.dma_start(w2_sb[:, t, :], w2[t * 128:(t + 1) * 128, :])
            o64_sb = pp.tile([D, 1], bf16)
            nc.sync.dma_start(o64_sb[:], o64[:])
            onr_sb = pp.tile([1, 128], bf16)
            nc.sync.dma_start(onr_sb[:], onr[:])

            # ---- persistent activations ----
            qk_all = pp.tile([128, NTOK], bf16)   # [qT; kT] raw
            qn_sb = pp.tile([D, NTOK], bf16)      # normalized qT
            kraw_sb = pp.tile([D, NTOK], bf16)    # raw kT at partitions 0-63
            vo_sb = pp.tile([128, NCH * JPC, D + 1], bf16)  # [v | ones]
            rtq_sb = pp.tile([1, NTOK], f32)      # sqrt(mean q^2) per token
            rks_sb = pp.tile([128, NCH * JPC], f32)  # 1/(|k|+8eps) per j
            nc.gpsimd.memset(vo_sb[:, :, D:D + 1], 1.0)

            cc_in = dram.tile([DIM, CH], bf16, name="cc_in")
            cc_out = dram.tile([DIM, CH], bf16, name="cc_out")

            stk_ps = ps.tile([128, NCH * JPC], f32, tag="stk", bufs=1)

            # ---- stage 1: projections + norm stats ----
            for ci in range(NCH):
                cols = slice(ci * CH, (ci + 1) * CH)
                xt = sb.tile([128, 4, CH], bf16, tag="xt")
                for t in range(4):
                    nc.sync.dma_start(xt[:, t, :],
                                      xT[t * 128:(t + 1) * 128, cols])
                qk_ps = ps.tile([128, CH], f32, tag="a", bufs=3)
                for t in range(4):
                    nc.tensor.matmul(qk_ps[:], wqk_sb[:, t, :], xt[:, t, :],
                                     start=(t == 0), stop=(t == 3))
                vt_ps = ps.tile([D, CH], f32, tag="b", bufs=2)
                for t in range(4):
                    nc.tensor.matmul(vt_ps[:], wv_sb[:, t, :], xt[:, t, :],
                                     start=(t == 0), stop=(t == 3))
                nc.vector.tensor_copy(qk_all[:, cols], qk_ps[:])
                # raw kT at partitions 0-63 (DMA partition remap)
                nc.sync.dma_start(kraw_sb[:, cols], qk_all[64:128, cols])
                # vT -> packed [j, d] tiles via DMA XBAR transpose
                vt_sb = sb.tile([D, CH], bf16, tag="vtsb")
                nc.vector.tensor_copy(vt_sb[:], vt_ps[:])
                for jj in range(JPC):
                    jt = ci * JPC + jj
                    js = slice(jj * 128, (jj + 1) * 128)
                    vtr = sb.tile([128, D], bf16, tag="vtr", bufs=4)
                    nc.sync.dma_start_transpose(vtr[:], vt_sb[:, js])
                    nc.gpsimd.tensor_copy(vo_sb[:, jt, 0:D], vtr[:])
                # k sum of squares per j (ones-matmul on remapped kraw)
                ksq = sb.tile([D, CH], bf16, tag="ksq")
                nc.vector.tensor_mul(ksq[:], kraw_sb[:, cols], kraw_sb[:, cols])
                for jj in range(JPC):
                    jt = ci * JPC + jj
                    js = slice(jj * 128, (jj + 1) * 128)
                    nc.tensor.matmul(stk_ps[:, jt:jt + 1], ksq[:, js],
                                     o64_sb[:], start=True, stop=True)
                # q sum of squares per token
                sq_q = sb.tile([D, CH], bf16, tag="sqq")
                nc.vector.tensor_mul(sq_q[:], qk_all[0:D, cols],
                                     qk_all[0:D, cols])
                stq_ps = ps.tile([1, CH], f32, tag="s", bufs=2)
                nc.tensor.matmul(stq_ps[:], o64_sb[:], sq_q[:],
                                 start=True, stop=True)
                nc.scalar.activation(rtq_sb[:, cols], stq_ps[:], AF.Sqrt,
                                     scale=1.0 / D)

                # k reciprocal per half (enables phase 2 of that batch)
                if ci in (3, 7):
                    h = ci // 4
                    hs = slice(h * JPB, (h + 1) * JPB)
                    skh = sb.tile([128, JPB], f32, tag="skh")
                    nc.scalar.activation(skh[:], stk_ps[:, hs], AF.Sqrt)
                    seh = sb.tile([128, JPB], f32, tag="seh")
                    nc.vector.tensor_scalar_add(seh[:], skh[:], 8.0 * EPS)
                    nc.vector.reciprocal(rks_sb[:, hs], seh[:])

            # ---- stage 1.5: batched q normalization ----
            req = sb.tile([1, NTOK], f32, tag="req")
            nc.vector.tensor_scalar_add(req[:], rtq_sb[:], EPS)
            rcq = sb.tile([1, NTOK], f32r, tag="rcq")
            nc.vector.reciprocal(rcq[:], req[:])
            rcq_b = sb.tile([1, NTOK], bf16, tag="rcqb")
            nc.vector.tensor_copy(rcq_b[:], rcq[:])
            for ci in range(NCH):
                cols = slice(ci * CH, (ci + 1) * CH)
                rb_ps = ps.tile([D, CH], f32, tag="b", bufs=2)
                nc.tensor.matmul(rb_ps[:], onr_sb[:, 0:D], rcq_b[:, cols],
                                 start=True, stop=True)
                rb_sb = sb.tile([D, CH], bf16, tag="rbsb")
                nc.vector.tensor_copy(rb_sb[:], rb_ps[:])
                nc.vector.tensor_mul(qn_sb[:, cols], qk_all[0:D, cols],
                                     rb_sb[:])

            # ---- phase 2: attention per 512-token i-chunk ----
            for b in range(B):
                for ch in range(NCH // B):
                    g = b * (NCH // B) + ch
                    i0 = g * CH
                    expT = sb.tile([128, JPB, CH], bf16, tag="exp")
                    for jt in range(JPB):
                        j0 = b * SEQ + jt * 128
                        sim_ps = ps.tile([128, CH], f32, tag="a", bufs=3)
                        nc.tensor.matmul(sim_ps[:], kraw_sb[:, j0:j0 + 128],
                                         qn_sb[:, i0:i0 + CH],
                                         start=True, stop=True)
                        gj = b * JPB + jt
                        nc.scalar.activation(expT[:, jt, :], sim_ps[:], AF.Exp,
                                             scale=rks_sb[:, gj:gj + 1])
                    av_ps = ps.tile([D + 1, CH], f32, tag="b", bufs=2)
                    for jt in range(JPB):
                        nc.tensor.matmul(av_ps[:], vo_sb[:, b * JPB + jt, :],
                                         expT[:, jt, :],
                                         start=(jt == 0), stop=(jt == JPB - 1))
                    rse = sb.tile([1, CH], f32r, tag="rse")
                    nc.vector.reciprocal(rse[:], av_ps[D:D + 1, :].bitcast(f32r))
                    rse_b = sb.tile([1, CH], bf16, tag="rseb")
                    nc.vector.tensor_copy(rse_b[:], rse[:])
                    r2_ps = ps.tile([D, CH], f32, tag="s", bufs=2)
                    nc.tensor.matmul(r2_ps[:], onr_sb[:, 0:D],
                                     rse_b[:], start=True, stop=True)
                    r2_sb = sb.tile([D, CH], f32, tag="r2sb")
                    nc.vector.tensor_copy(r2_sb[:], r2_ps[:])
                    oc = sb.tile([D, CH], bf16, tag="oc")
                    nc.vector.tensor_mul(oc[:], av_ps[0:D, :], r2_sb[:])
                    nc.sync.dma_start(cc_in[g * D:(g + 1) * D, :], oc[:])

            # ---- shard transpose: one AllToAll ----
            if collective:
                nc.gpsimd.collective_compute(
                    "AllToAll", ALU.bypass,
                    replica_groups=[list(range(num_devices))],
                    ins=[cc_in[:]], outs=[cc_out[:]])
            else:
                # timing-only stand-in (numerically wrong off-diagonal)
                nc.sync.dma_start(cc_out[:], cc_in[:])

            # ---- token-parallel out-projection ----
            ag = sb.tile([128, 4, CH], bf16, tag="ag")
            for t in range(4):
                nc.sync.dma_start(ag[:, t, :], cc_out[t * 128:(t + 1) * 128, :])
            for mt in range(4):
                fp_ps = ps.tile([128, CH], f32, tag="a", bufs=3)
                for t in range(4):
                    nc.tensor.matmul(fp_ps[:],
                                     w2_sb[:, t, mt * 128:(mt + 1) * 128],
                                     ag[:, t, :], start=(t == 0), stop=(t == 3))
                fo = sb.tile([128, CH], f32, tag="fo")
                nc.vector.tensor_copy(fo[:], fp_ps[:])
                nc.sync.dma_start(outT[mt * 128:(mt + 1) * 128, :], fo[:])
    nc.compile()
    _BUILD_CACHE[key] = nc
    return nc


def make_in_maps(x, Wq, Wkv, Wout):
    xT = np.ascontiguousarray(
        x.reshape(NTOK, DIM).T).astype(ml_dtypes.bfloat16)
    w2 = np.ascontiguousarray(Wout.T).astype(ml_dtypes.bfloat16)
    o64 = np.ones((D, 1), ml_dtypes.bfloat16)
    onr = np.ones((1, 128), ml_dtypes.bfloat16)
    in_maps = []
    for c in range(N_CORES):
        rows = slice(c * D, (c + 1) * D)
        wqk = np.ascontiguousarray(
            np.concatenate([Wq[rows, :].T, Wkv[rows, :].T],
                           axis=1)).astype(ml_dtypes.bfloat16)
        wv = np.ascontiguousarray(
            Wkv[DIM + c * D:DIM + (c + 1) * D, :].T).astype(ml_dtypes.bfloat16)
        in_maps.append({
            "xT": xT, "wqk": wqk, "wv": wv, "w2": w2,
            "o64": o64, "onr": onr,
        })
    return in_maps


def kernel(x, Wq, Wkv, Wout, _trace=False, _collective=True):
    nc = build(collective=_collective)
    in_maps = make_in_maps(np.asarray(x), np.asarray(Wq), np.asarray(Wkv),
                           np.asarray(Wout))
    res = bass_utils.run_bass_kernel_spmd(
        nc, in_maps, core_ids=list(range(N_CORES)), trace=_trace)
    out = np.empty((NTOK, DIM), np.float32)
    for c in range(N_CORES):
        out[c * CH:(c + 1) * CH, :] = res.results[c]["outT"].T
    full = out.reshape(B, SEQ, DIM)
    if _trace:
        return full, res
    return full
